# revision 5
# baseline (speedup 1.0000x reference)
"""Trainium2 Bass kernel for nn_BernMLPAugmenter (GNN edge-MLP augmenter).

Math (reference):
    edge_emb = concat(node_emb[src], node_emb[dst])        # [M, 256]
    h        = relu(edge_emb @ W1 + b1)                    # [M, 64]
    logit    = h @ W2 + b2                                 # [M, 1]
    eps      = (2b-1)*eps_raw + (1-b);  g = ln(eps) - ln(1-eps)
    w        = sigmoid((g + logit)/T);  return (concat(w, w), w)

Device algorithm (8 NeuronCores, SPMD, full inputs in / full outputs out):
  Since concat(es,ed)@W1 = es@W1[:128] + ed@W1[128:], precompute per-node
  projections once (phase 0), then the per-edge work is a gather plus cheap
  vector ops (phase 1).  W2 is folded into the tables column-wise by |w2|
  with a sign-split (relu(x)*w2 = sign * relu(|w2|*x)), so the per-edge dot
  with W2 becomes two segmented reductions.

  Phase 0 (each core): table[n] = [node_emb[n] @ W1a' | node_emb[n] @ W1b' + b1']
  (128 f32/row, 512B) written to a DRAM table laid out [128, T, 128] so row
  rho(n) = (n%128)*T + n//128 is 512B-contiguous.  PE matmuls, DVE/ACT copies.

  Phase 1: edges are binned on the host into a 4x4 grid of (src-quarter,
  dst-quarter) cells by node id mod 128 (quarter = (n%128)//32); each core
  gets 2 cells.  Within a cell both endpoints live in a 25024-row table
  region, so local indices fit int16 and the fast GPSIMD dma_gather ucode
  (<=1024 idxs/call) fetches 256B half-rows.  The per-core node column
  layout is permuted on the host so every core finds its cells' quarters at
  the same fixed regions -> one uniform SPMD program.

  h = relu(gs + gd) on DVE/ACT; logits via two segmented X-reductions
  (sign-split); Gumbel-sigmoid gate on ACT (Ln / Sigmoid with fused affine).
"""
import math
import numpy as np

from concourse import bacc, mybir
from concourse.tile import TileContext
from concourse.bass_utils import run_bass_kernel_spmd

F32 = mybir.dt.float32
I16 = mybir.dt.int16

N_CORES = 8
EMB = 128
MLP = 64
EPS_BIAS = 1e-4
TEMP = 1.0


class Cfg:
    def __init__(self, n_tiles, e_pad, pos_cnt, b1_zero, b2val, sc=8192, gn=1024,
                 slab=16, n_queues=1, trace=False, reps=1):
        self.n_tiles = n_tiles            # 128-node tiles; table rows = 128*n_tiles
        self.nrows = 128 * n_tiles
        self.qrows = 32 * n_tiles         # rows per region (quarter)
        self.e_pad = e_pad                # padded edges per cell (mult of sc)
        self.pos_cnt = pos_cnt            # of 64 MLP cols, how many have w2 >= 0
        self.b1_zero = b1_zero
        self.b2val = b2val
        self.sc = sc                      # compute chunk (edges)
        self.gn = gn                      # edges per dma_gather call (<=1024)
        self.slab = slab                  # node tiles per phase-0 slab
        self.n_queues = n_queues
        self.trace = trace
        self.reps = reps

    def key(self):
        return (self.n_tiles, self.e_pad, self.pos_cnt, self.b1_zero,
                float(self.b2val), self.sc, self.gn, self.slab, self.n_queues,
                self.reps)


def build_program(cfg: Cfg):
    nc = bacc.Bacc(num_swdge_queues=cfg.n_queues)
    NT = cfg.n_tiles
    T = cfg.e_pad // 128                  # output cols per cell
    n_chunks = cfg.e_pad // cfg.sc
    subg = cfg.sc // cfg.gn               # gathers per stream per chunk
    blk = cfg.sc // 128                   # 64-elem blocks per chunk

    node_t = nc.declare_dram_parameter("node_t", [128, NT * 128], F32, isOutput=False)
    w1ab = nc.declare_dram_parameter("w1ab", [128, 128], F32, isOutput=False)
    if not cfg.b1_zero:
        b1rep = nc.declare_dram_parameter("b1rep", [128, 4 * 128], F32, isOutput=False)
    cells = []
    for j in range(2):
        cells.append(dict(
            isrc=nc.declare_dram_parameter(f"isrc{j}", [128, cfg.e_pad // 16], I16, isOutput=False),
            idst=nc.declare_dram_parameter(f"idst{j}", [128, cfg.e_pad // 16], I16, isOutput=False),
            eps=nc.declare_dram_parameter(f"eps{j}", [128, T], F32, isOutput=False),
            wout=nc.declare_dram_parameter(f"wout{j}", [128, T], F32, isOutput=True),
        ))

    table = nc.dram_tensor("ptab", [128, NT, 128], F32)
    tab_rows = table[:].rearrange("p t d -> (p t) d")

    with TileContext(nc) as tc:
        import contextlib
        loop_cm = tc.For_i(0, cfg.reps, 1) if cfg.reps > 1 else contextlib.nullcontext()
        with loop_cm, tc.tile_pool(name="const", bufs=1) as cpool:
            w1ab_t = cpool.tile([128, 128], F32)
            nc.sync.dma_start(out=w1ab_t[:], in_=w1ab[:])
            bias_u = cpool.tile([128, 1], F32)
            nc.vector.memset(bias_u[:], 1.0 - EPS_BIAS)
            bias_v = cpool.tile([128, 1], F32)
            nc.vector.memset(bias_v[:], EPS_BIAS)
            bias_b2 = cpool.tile([128, 1], F32)
            nc.vector.memset(bias_b2[:], cfg.b2val / TEMP)
            if not cfg.b1_zero:
                b1t = cpool.tile([128, 4 * 128], F32)
                nc.sync.dma_start(out=b1t[:], in_=b1rep[:])

            # ---------------- phase 0: build projection table ----------------
            with tc.tile_pool(name="p0in", bufs=3) as ip, \
                 tc.tile_pool(name="p0out", bufs=2) as op, \
                 tc.tile_pool(name="p0ps", bufs=8, space="PSUM") as pp:
                copy_tick = 0
                t0 = 0
                while t0 < NT:
                    nt = min(cfg.slab, NT - t0)
                    slab_t = ip.tile([128, cfg.slab * 128], F32, tag="slab")
                    nc.sync.dma_start(out=slab_t[:, :nt * 128],
                                      in_=node_t[:, t0 * 128:(t0 + nt) * 128])
                    ob = op.tile([128, cfg.slab * 128], F32, tag="ob")
                    for g0 in range(0, nt, 4):
                        gnn = min(4, nt - g0)
                        ps = pp.tile([128, 512], F32, tag="ps")
                        for j in range(gnn):
                            nc.tensor.matmul(
                                out=ps[:, j * 128:(j + 1) * 128],
                                lhsT=slab_t[:, (g0 + j) * 128:(g0 + j + 1) * 128],
                                rhs=w1ab_t[:],
                                start=True, stop=True)
                        dst = ob[:, g0 * 128:(g0 + gnn) * 128]
                        if cfg.b1_zero:
                            if copy_tick % 2 == 0:
                                nc.vector.tensor_copy(out=dst, in_=ps[:, :gnn * 128])
                            else:
                                nc.scalar.copy(out=dst, in_=ps[:, :gnn * 128])
                            copy_tick += 1
                        else:
                            nc.vector.tensor_tensor(out=dst, in0=ps[:, :gnn * 128],
                                                    in1=b1t[:, :gnn * 128],
                                                    op=mybir.AluOpType.add)
                    nc.sync.dma_start(out=table[:, t0:t0 + nt, :], in_=ob[:, :nt * 128])
                    t0 += nt

            # ---------------- phase 1: per-edge gather + MLP + gate ----------------
            with tc.tile_pool(name="idx", bufs=1) as xp, \
                 tc.tile_pool(name="gath", bufs=2) as gp, \
                 tc.tile_pool(name="hbuf", bufs=2) as hp, \
                 tc.tile_pool(name="red", bufs=3) as rp, \
                 tc.tile_pool(name="smal", bufs=1) as sp:
                qq = 0
                for j, cell in enumerate(cells):
                    # src always region 0; dst region 1 (cell 0) / region 3 (cell 1)
                    src_base = 0
                    dst_base = (1 if j == 0 else 3) * cfg.qrows
                    isrc_t = xp.tile([128, cfg.e_pad // 16], I16, tag=f"isrc{j}")
                    nc.sync.dma_start(out=isrc_t[:], in_=cell["isrc"][:])
                    idst_t = xp.tile([128, cfg.e_pad // 16], I16, tag=f"idst{j}")
                    nc.sync.dma_start(out=idst_t[:], in_=cell["idst"][:])
                    logits = sp.tile([128, T], F32, tag=f"logits{j}")

                    for c in range(n_chunks):
                        gs = gp.tile([128, blk * 64], F32, tag="gs")
                        gd = gp.tile([128, blk * 64], F32, tag="gd")
                        gs3 = gs[:].rearrange("p (b d) -> p b d", d=64)
                        gd3 = gd[:].rearrange("p (b d) -> p b d", d=64)
                        for s in range(subg):
                            i0 = c * cfg.sc + s * cfg.gn
                            col0 = i0 // 16
                            ncols = cfg.gn // 16
                            b0 = s * (cfg.gn // 128)
                            b1_ = (s + 1) * (cfg.gn // 128)
                            nc.gpsimd.dma_gather(
                                out_ap=gs3[:, b0:b1_, :],
                                in_ap=tab_rows[src_base:src_base + cfg.qrows, 0:64],
                                idxs_ap=isrc_t[:, col0:col0 + ncols],
                                num_idxs=cfg.gn, num_idxs_reg=cfg.gn,
                                elem_size=64, elem_step=128,
                                queue_num=qq % cfg.n_queues)
                            qq += 1
                            nc.gpsimd.dma_gather(
                                out_ap=gd3[:, b0:b1_, :],
                                in_ap=tab_rows[dst_base:dst_base + cfg.qrows, 64:128],
                                idxs_ap=idst_t[:, col0:col0 + ncols],
                                num_idxs=cfg.gn, num_idxs_reg=cfg.gn,
                                elem_size=64, elem_step=128,
                                queue_num=qq % cfg.n_queues)
                            qq += 1
                        y = hp.tile([128, blk * 64], F32, tag="y")
                        nc.vector.tensor_tensor(out=y[:], in0=gs[:], in1=gd[:],
                                                op=mybir.AluOpType.add)
                        h = hp.tile([128, blk * 64], F32, tag="h")
                        nc.scalar.activation(out=h[:], in_=y[:],
                                             func=mybir.ActivationFunctionType.Relu)
                        h3 = h[:].rearrange("p (b d) -> p b d", d=64)
                        lslice = logits[:, c * blk:(c + 1) * blk]
                        if cfg.pos_cnt == 64:
                            nc.vector.reduce_sum(out=lslice, in_=h3[:, :, :],
                                                 axis=mybir.AxisListType.X)
                        elif cfg.pos_cnt == 0:
                            rn = rp.tile([128, blk], F32, tag="rn")
                            nc.vector.reduce_sum(out=rn[:], in_=h3[:, :, :],
                                                 axis=mybir.AxisListType.X)
                            nc.vector.tensor_scalar_mul(out=lslice, in0=rn[:], scalar1=-1.0)
                        else:
                            rpos = rp.tile([128, blk], F32, tag="rp")
                            nc.vector.reduce_sum(out=rpos[:], in_=h3[:, :, :cfg.pos_cnt],
                                                 axis=mybir.AxisListType.X)
                            rn = rp.tile([128, blk], F32, tag="rn")
                            nc.vector.reduce_sum(out=rn[:], in_=h3[:, :, cfg.pos_cnt:],
                                                 axis=mybir.AxisListType.X)
                            nc.vector.tensor_sub(out=lslice, in0=rpos[:], in1=rn[:])

                    # gate: w = sigmoid((ln(u) - ln(v) + logits)/T + b2/T)
                    eps_t = sp.tile([128, T], F32, tag=f"eps{j}")
                    nc.sync.dma_start(out=eps_t[:], in_=cell["eps"][:])
                    lnu = sp.tile([128, T], F32, tag=f"lnu{j}")
                    nc.scalar.activation(out=lnu[:], in_=eps_t[:],
                                         func=mybir.ActivationFunctionType.Ln,
                                         scale=2.0 * EPS_BIAS - 1.0, bias=bias_u[:, :1])
                    lnv = sp.tile([128, T], F32, tag=f"lnv{j}")
                    nc.scalar.activation(out=lnv[:], in_=eps_t[:],
                                         func=mybir.ActivationFunctionType.Ln,
                                         scale=1.0 - 2.0 * EPS_BIAS, bias=bias_v[:, :1])
                    gg = sp.tile([128, T], F32, tag=f"gg{j}")
                    nc.vector.tensor_sub(out=gg[:], in0=lnu[:], in1=lnv[:])
                    nc.vector.tensor_tensor(out=gg[:], in0=gg[:], in1=logits[:],
                                            op=mybir.AluOpType.add)
                    wt = sp.tile([128, T], F32, tag=f"wt{j}")
                    nc.scalar.activation(out=wt[:], in_=gg[:],
                                         func=mybir.ActivationFunctionType.Sigmoid,
                                         scale=1.0 / TEMP, bias=bias_b2[:, :1])
                    nc.sync.dma_start(out=cell["wout"][:], in_=wt[:])
    nc.compile()
    return nc


def _wrap_idx(flat):
    """int16 stream -> [128, E/16] wrapped-in-16-partitions, replicated x8."""
    e = flat.shape[0]
    a = flat.reshape(e // 16, 16).T.astype(np.int16)
    return np.ascontiguousarray(np.tile(a, (8, 1)))


def _col_major(flat, dtype):
    """stream i -> [128, E/128] with element i at [i%128, i//128]."""
    e = flat.shape[0]
    return np.ascontiguousarray(flat.reshape(e // 128, 128).T.astype(dtype))


_PROG_CACHE = {}


def _prep_and_run(node_emb, edge_index, eps_raw, W1, b1, W2, b2, cfg_over=None):
    M = eps_raw.shape[0]
    n_nodes = node_emb.shape[0]
    n_tiles = (n_nodes + 127) // 128
    NT = n_tiles

    src = edge_index[0, :M].astype(np.int64)
    dst = edge_index[1, :M].astype(np.int64)

    w2 = np.asarray(W2[:, 0], dtype=np.float64)
    sgn_neg = w2 < 0
    perm_f = np.argsort(sgn_neg, kind="stable")      # w2>=0 cols first
    pos_cnt = int((~sgn_neg).sum())
    aw2 = np.abs(w2)[perm_f]

    W1a = np.asarray(W1[:EMB], dtype=np.float64)[:, perm_f] * aw2[None, :]
    W1b = np.asarray(W1[EMB:], dtype=np.float64)[:, perm_f] * aw2[None, :]
    w1ab = np.concatenate([W1a, W1b], axis=1).astype(np.float32)  # [128, 128]
    b1s = (np.asarray(b1, dtype=np.float64)[perm_f] * aw2).astype(np.float32)
    b1_zero = bool(np.all(b1s == 0.0))
    b1row = np.concatenate([np.zeros(64, np.float32), b1s])
    b1rep = np.tile(np.tile(b1row, 4)[None, :], (128, 1)).astype(np.float32)
    b2val = float(np.asarray(b2).ravel()[0])

    # ---- bin edges into 16 (src-quarter, dst-quarter) cells ----
    qs = (src & 127) >> 5
    qd = (dst & 127) >> 5
    cell = qs * 4 + qd
    order = np.argsort(cell, kind="stable")
    counts = np.bincount(cell, minlength=16)
    offs = np.zeros(17, np.int64)
    np.cumsum(counts, out=offs[1:])

    sc = cfg_over.get("sc", 8192) if cfg_over else 8192
    gn = cfg_over.get("gn", 1024) if cfg_over else 1024
    e_pad = max(int(math.ceil(counts.max() / sc)) * sc, sc)

    cfg = Cfg(n_tiles=NT, e_pad=e_pad, pos_cnt=pos_cnt, b1_zero=b1_zero,
              b2val=b2val, sc=sc, gn=gn,
              n_queues=cfg_over.get("n_queues", 1) if cfg_over else 1,
              reps=cfg_over.get("reps", 1) if cfg_over else 1)
    key = cfg.key()
    if key not in _PROG_CACHE:
        _PROG_CACHE[key] = build_program(cfg)
    nc = _PROG_CACHE[key]

    # ---- per-core inputs ----
    embT = np.zeros((128, NT * 128), dtype=np.float32)
    embT[:, :n_nodes] = np.ascontiguousarray(node_emb.T)

    in_maps = []
    cell_pos = {}
    for core in range(N_CORES):
        k0, k1 = 2 * core, 2 * core + 1
        qs0 = k0 // 4                   # shared src quarter of both cells
        qd0, qd1 = k0 % 4, k1 % 4
        REG = [qs0, qd0, qs0, qd1]      # quarter hosted by each partition group
        permp = np.empty(128, np.int64)
        for r in range(4):
            permp[32 * r:32 * r + 32] = 32 * REG[r] + np.arange(32)
        cols = (np.arange(NT)[:, None] * 128 + permp[None, :]).ravel()
        nt_core = np.ascontiguousarray(embT[:, cols])

        m = {"node_t": nt_core, "w1ab": w1ab}
        if not b1_zero:
            m["b1rep"] = b1rep
        for j, k in enumerate((k0, k1)):
            pos = order[offs[k]:offs[k + 1]]
            cell_pos[k] = pos
            e_cell = pos.shape[0]
            s_loc = ((src[pos] & 127) - 32 * (k // 4)) * NT + (src[pos] >> 7)
            d_loc = ((dst[pos] & 127) - 32 * (k % 4)) * NT + (dst[pos] >> 7)
            s_loc = np.concatenate([s_loc, np.zeros(e_pad - e_cell, np.int64)])
            d_loc = np.concatenate([d_loc, np.zeros(e_pad - e_cell, np.int64)])
            ep = np.full(e_pad, 0.5, np.float32)
            ep[:e_cell] = eps_raw[pos, 0]
            m[f"isrc{j}"] = _wrap_idx(s_loc)
            m[f"idst{j}"] = _wrap_idx(d_loc)
            m[f"eps{j}"] = _col_major(ep, np.float32)
        in_maps.append(m)

    do_trace = bool(cfg_over and cfg_over.get("trace"))
    res = run_bass_kernel_spmd(
        nc, in_maps, list(range(N_CORES)), trace=do_trace,
        trace_cores=list(range(N_CORES)) if do_trace else None)

    w_full = np.zeros(M, dtype=np.float32)
    for core in range(N_CORES):
        for j, k in enumerate((2 * core, 2 * core + 1)):
            pos = cell_pos[k]
            wt = res.results[core][f"wout{j}"]
            w_full[pos] = wt.T.ravel()[:pos.shape[0]]
    return w_full, res


def kernel(node_emb, edge_index, eps_raw, W1, b1, W2, b2):
    w, _ = _prep_and_run(np.asarray(node_emb), np.asarray(edge_index),
                         np.asarray(eps_raw), np.asarray(W1), np.asarray(b1),
                         np.asarray(W2), np.asarray(b2))
    return (np.concatenate([w, w]), w)
